# revision 17
# baseline (speedup 1.0000x reference)
"""Trainium2 Bass kernel for the diagonal complex linear recurrence (SSM scan).

Problem: out[t, d] = z_d * out[t-1, d] + x[t, d],  z_d = exp(-exp(size_d) + i*theta_d)
         x: [T=8192, D=2048] f32, out: [T, D] complex64.

Strategy (v3, fp16):
  - Shard channels D across 8 cores (256 each), pure model parallelism.
  - Per core, layout [channels(partitions), time(free)].  Per time-chunk of
    length L the complex scan splits via a local phase twist into two REAL
    first-order scans (hardware tensor_tensor_scan, 2.03 cyc/elem):
        v[jL+l] = e^{i*theta*l} * W_j[l]
        W_j[l]  = r * W_j[l-1] + e^{-i*theta*l} * x[jL+l],   r = |z|
  - fp16 everywhere for the DVE 2x packed mode; the scan decay operand is a
    stride-0 broadcast of r [128,1] fp32 (exact r^k chain).
  - Sign trick: scan the negated imag chain (u_im' = +x*sin => W_im' = -W_im):
        v_re = cos.W_re + sin.W_im'   (eye, eye)
        v_im = sin.W_re - cos.W_im'   (eye, eyeNEG)
    combines run on the PE as identity matmul accumulation into PSUM;
    ScalarE copies PSUM->SBUF (fp32->fp16).
  - W_re/W_im' live in one contiguous tile [128, 2L]; with paired tables
    (cos||sin) and (sin||cos) each untwist needs just 2 full-length TTs:
        tA = (cos||sin) . (Wre||Wim')  = t1 || t2
        tB = (sin||cos) . (Wre||Wim')  = t3 || t4
  - First chunk is split so compute starts during the table DMA; the last
    chunk is split so the scan->untwist->PE->copy->DMA tail overlaps.
"""

import os
import sys

import numpy as np

for _p in ("/opt/trn_rl_repo", "/root/.axon_site/_ro/trn_rl_repo"):
    if os.path.isdir(_p) and _p not in sys.path:
        sys.path.append(_p)

import concourse.bacc as bacc
import concourse.mybir as mybir
from concourse import bass_utils
from concourse.tile import TileContext

T = 8192
D = 2048
NCORES = 8
DS = D // NCORES          # 256 channels per core
G = DS // 128             # partition groups per core (2)
L = 2048                  # twist-chunk length (scan segment)
C = T // L                # chunks (4)
F32 = mybir.dt.float32
F16 = mybir.dt.float16

_PROGRAM = None


def _build_program():
    nc = bacc.Bacc("TRN2", target_bir_lowering=False)

    xT = nc.dram_tensor("xT", (DS, T), F16, kind="ExternalInput")
    # paired twist table: [cos(th*l) || sin(th*l)]
    cossin = nc.dram_tensor("cossin", (DS, 2 * L), F16, kind="ExternalInput")
    rb = nc.dram_tensor("rb", (DS, 1), F32, kind="ExternalInput")
    bnd = nc.dram_tensor("bnd", (DS, 4), F32, kind="ExternalInput")  # cL,sL,nsL,0
    eye = nc.dram_tensor("eye", (128, 256), F16, kind="ExternalInput")  # [I, -I]
    out_re = nc.dram_tensor("out_re", (DS, T), F16, kind="ExternalOutput")
    out_im = nc.dram_tensor("out_im", (DS, T), F16, kind="ExternalOutput")

    mult = mybir.AluOpType.mult
    add = mybir.AluOpType.add
    ident = mybir.ActivationFunctionType.Identity
    MMF = 512  # per-matmul free dim (one PSUM bank)

    # work pieces: (chunk, lo, hi) with twist-phase l = local offset in chunk.
    # chunk 0 laddered for DMA warm-up; last chunk split to shorten the tail.
    L0 = 256
    L1 = 1024
    pieces = [(0, 0, L0), (0, L0, L1), (0, L1, L)]
    for j in range(1, C - 1):
        pieces.append((j, 0, L))
    pieces += [(C - 1, 0, L // 2), (C - 1, L // 2, 7 * L // 8),
               (C - 1, 7 * L // 8, L)]

    with TileContext(nc) as tc:
        with tc.tile_pool(name="tabs", bufs=1) as tpool, \
             tc.tile_pool(name="xp", bufs=2) as xpool, \
             tc.tile_pool(name="work", bufs=2) as pool, \
             tc.tile_pool(name="outp", bufs=3) as opool, \
             tc.tile_pool(name="kpool", bufs=4) as kpool, \
             tc.tile_pool(name="psum", bufs=2, space="PSUM") as ppool:
            # small constants first, then a ladder of table/x lead-ins so the
            # DVE starts after ~0.5 MB of DMA instead of the full tables.
            tabs = []
            x0 = []
            for g in range(G):
                pg = slice(g * 128, (g + 1) * 128)
                rb_t = tpool.tile([128, 1], F32, name=f"rb_t{g}")
                bnd_t = tpool.tile([128, 4], F32, name=f"bnd_t{g}")
                cs_t = tpool.tile([128, 2 * L], F16, name=f"cs_t{g}")
                sc_t = tpool.tile([128, 2 * L], F16, name=f"sc_t{g}")
                xt = xpool.tile([128, L], F16, name="xt", tag="xt")
                nc.sync.dma_start(rb_t[:], rb[pg, :])
                nc.sync.dma_start(bnd_t[:], bnd[pg, :])
                nc.sync.dma_start(xt[:, 0:L0], xT[pg, 0:L0])
                nc.sync.dma_start(cs_t[:, 0:L0], cossin[pg, 0:L0])
                nc.sync.dma_start(cs_t[:, L:L + L0], cossin[pg, L:L + L0])
                tabs.append([cs_t, sc_t, rb_t, bnd_t])
                x0.append(xt)
            eye_t = tpool.tile([128, 256], F16, name="eye_t")
            nc.sync.dma_start(eye_t[:], eye[:])
            # second rung of the ladder, then the bulk
            for g in range(G):
                pg = slice(g * 128, (g + 1) * 128)
                cs_t = tabs[g][0]
                nc.sync.dma_start(x0[g][:, L0:L1], xT[pg, L0:L1])
                nc.sync.dma_start(cs_t[:, L0:L1], cossin[pg, L0:L1])
                nc.sync.dma_start(cs_t[:, L + L0:L + L1],
                                  cossin[pg, L + L0:L + L1])
            for g in range(G):
                pg = slice(g * 128, (g + 1) * 128)
                cs_t = tabs[g][0]
                nc.sync.dma_start(x0[g][:, L1:L], xT[pg, L1:L])
                nc.sync.dma_start(cs_t[:, L1:L], cossin[pg, L1:L])
                nc.sync.dma_start(cs_t[:, L + L1:2 * L],
                                  cossin[pg, L + L1:2 * L])
            # build the swapped table (sin||cos) on ScalarE once per group,
            # so tB is a single full-length TT on full chunks.
            for g in range(G):
                cs_t, sc_t = tabs[g][0], tabs[g][1]
                nc.scalar.copy(sc_t[:, 0:L], cs_t[:, L:2 * L])
                nc.scalar.copy(sc_t[:, L:2 * L], cs_t[:, 0:L])
            eyeP = eye_t[:, 0:128]
            eyeN = eye_t[:, 128:256]

            K = [[None, None] for _ in range(G)]   # fp32 carries per group
            cur = [None] * G                       # (xt, wri) per group
            for (j, a, b) in pieces:
                for g in range(G):
                    pg = slice(g * 128, (g + 1) * 128)
                    cs_t, sc_t, rb_t, bnd_t = tabs[g]
                    ts = slice(j * L + a, j * L + b)
                    n = b - a

                    if a == 0:
                        if j == 0:
                            xt = x0[g]
                        else:
                            xt = xpool.tile([128, L], F16, name="xt", tag="xt")
                            nc.sync.dma_start(xt[:], xT[pg, j * L:(j + 1) * L])
                        # W_re || W_im' in one contiguous tile [128, 2L]
                        wri = pool.tile([128, 2 * L], F16, name="wri",
                                        tag="wri")
                        cur[g] = (xt, wri)
                    else:
                        xt, wri = cur[g]
                    wre = wri[:, 0:L]
                    wim = wri[:, L:2 * L]

                    # twist: u_re = x*cos, u_im' = x*sin   (fp16 2x)
                    ure = pool.tile([128, n], F16, name="ure", tag="ure")
                    uim = pool.tile([128, n], F16, name="uim", tag="uim")
                    nc.vector.tensor_tensor(ure[:], xt[:, a:b],
                                            cs_t[:, a:b], op=mult)
                    nc.vector.tensor_tensor(uim[:], xt[:, a:b],
                                            cs_t[:, L + a:L + b], op=mult)

                    # real scans with decay r (stride-0 broadcast, fp32 chain)
                    rbb = rb_t[:].broadcast_to((128, n))
                    if a == 0:
                        init_re = 0.0 if j == 0 else K[g][0][:]
                        init_im = 0.0 if j == 0 else K[g][1][:]
                    else:
                        init_re = wre[:, a - 1:a]
                        init_im = wim[:, a - 1:a]
                    nc.vector.tensor_tensor_scan(
                        wre[:, a:b], rbb, ure[:], init_re, op0=mult, op1=add)
                    nc.vector.tensor_tensor_scan(
                        wim[:, a:b], rbb, uim[:], init_im, op0=mult, op1=add)

                    # carry rotation for next chunk (ScalarE, fp32):
                    #   Kre = cL*WreE + sL*WimE';  Kim' = cL*WimE' - sL*WreE
                    if b == L and j < C - 1:
                        cL, sL = bnd_t[:, 0:1], bnd_t[:, 1:2]
                        nsL = bnd_t[:, 2:3]
                        wreE, wimE = wre[:, L - 1:L], wim[:, L - 1:L]
                        tmp1 = kpool.tile([128, 1], F32, name="tmp1", tag="t1")
                        tmp2 = kpool.tile([128, 1], F32, name="tmp2", tag="t2")
                        kre = kpool.tile([128, 1], F32, name="kre", tag="kre")
                        kim = kpool.tile([128, 1], F32, name="kim", tag="kim")
                        nc.scalar.activation(tmp1[:], wreE, ident, scale=cL)
                        nc.scalar.activation(kre[:], wimE, ident,
                                             scale=sL, bias=tmp1[:])
                        nc.scalar.activation(tmp2[:], wreE, ident, scale=nsL)
                        nc.scalar.activation(kim[:], wimE, ident,
                                             scale=cL, bias=tmp2[:])
                        K[g][0], K[g][1] = kre, kim

                    # untwist (fp16 2x):
                    #   tA = (cos||sin) . (Wre||Wim') = t1 || t2
                    #   tB = (sin . Wre) || (cos . Wim') = t3 || t4
                    # tA is one full-length TT when the piece spans the chunk.
                    tA = pool.tile([128, 2 * n], F16, name="tA", tag="tA")
                    tB = pool.tile([128, 2 * n], F16, name="tB", tag="tB")
                    if a == 0 and b == L:
                        nc.vector.tensor_tensor(
                            tA[:], cs_t[:], wri[:], op=mult)
                        nc.vector.tensor_tensor(
                            tB[:], sc_t[:], wri[:], op=mult)
                    else:
                        nc.vector.tensor_tensor(
                            tA[:, 0:n], cs_t[:, a:b], wre[:, a:b], op=mult)
                        nc.vector.tensor_tensor(
                            tA[:, n:2 * n], cs_t[:, L + a:L + b],
                            wim[:, a:b], op=mult)
                        nc.vector.tensor_tensor(
                            tB[:, 0:n], cs_t[:, L + a:L + b], wre[:, a:b],
                            op=mult)
                        nc.vector.tensor_tensor(
                            tB[:, n:2 * n], cs_t[:, a:b], wim[:, a:b],
                            op=mult)

                    # combines on PE: psum_re = t1 + t2 ; psum_im = t3 - t4
                    ore = opool.tile([128, n], F16, name="ore", tag="ore")
                    oim = opool.tile([128, n], F16, name="oim", tag="oim")
                    for q in range(0, n, 1024):
                        hh = min(1024, n - q)
                        pre = ppool.tile([128, hh], F32, name="pre", tag="pre")
                        pim = ppool.tile([128, hh], F32, name="pim", tag="pim")
                        for h in range(0, hh, MMF):
                            w = min(MMF, hh - h)
                            hs = slice(q + h, q + h + w)
                            hs2 = slice(n + q + h, n + q + h + w)
                            ps = slice(h, h + w)
                            nc.tensor.matmul(pre[:, ps], eyeP, tA[:, hs],
                                             start=True, stop=False)
                            nc.tensor.matmul(pre[:, ps], eyeP, tA[:, hs2],
                                             start=False, stop=True)
                            nc.tensor.matmul(pim[:, ps], eyeP, tB[:, hs],
                                             start=True, stop=False)
                            nc.tensor.matmul(pim[:, ps], eyeN, tB[:, hs2],
                                             start=False, stop=True)
                        qs = slice(q, q + hh)
                        nc.scalar.copy(ore[:, qs], pre[:])
                        nc.scalar.copy(oim[:, qs], pim[:])
                    nc.sync.dma_start(out_re[pg, ts], ore[:])
                    nc.sync.dma_start(out_im[pg, ts], oim[:])

    nc.compile()
    return nc


def _get_program():
    global _PROGRAM
    if _PROGRAM is None:
        _PROGRAM = _build_program()
    return _PROGRAM


def _host_prep(x, size, theta):
    """Per-core input maps (host-side sharding + table precompute)."""
    size64 = np.asarray(size, np.float64)
    theta64 = np.asarray(theta, np.float64)
    r64 = np.exp(-np.exp(size64))                      # [D]
    l64 = np.arange(L, dtype=np.float64)
    ang = theta64[:, None] * l64[None, :]              # [D, L]
    cosl = np.cos(ang).astype(np.float16)
    sinl = np.sin(ang).astype(np.float16)
    cossin = np.concatenate([cosl, sinl], axis=1)      # [D, 2L]
    rbf = r64.astype(np.float32)[:, None]
    bnd = np.zeros((D, 4), np.float32)
    bnd[:, 0] = np.cos(theta64 * L)
    bnd[:, 1] = np.sin(theta64 * L)
    bnd[:, 2] = -np.sin(theta64 * L)

    xh = np.asarray(x, np.float16)
    eye128 = np.eye(128, dtype=np.float16)
    eye = np.concatenate([eye128, -eye128], axis=1)    # [128, 256]
    in_maps = []
    for c in range(NCORES):
        sl = slice(c * DS, (c + 1) * DS)
        in_maps.append({
            "xT": np.ascontiguousarray(xh[:, sl].T),
            "cossin": np.ascontiguousarray(cossin[sl]),
            "rb": np.ascontiguousarray(rbf[sl]),
            "bnd": np.ascontiguousarray(bnd[sl]),
            "eye": eye,
        })
    return in_maps


def _assemble(results):
    out = np.empty((T, D), np.complex64)
    for c, res in enumerate(results):
        sl = slice(c * DS, (c + 1) * DS)
        out[:, sl] = (res["out_re"].astype(np.float32)
                      + 1j * res["out_im"].astype(np.float32)).T
    return out


def run(x, size, theta, trace=False, **spmd_kwargs):
    nc = _get_program()
    in_maps = _host_prep(x, size, theta)
    res = bass_utils.run_bass_kernel_spmd(
        nc, in_maps, core_ids=list(range(NCORES)), trace=trace, **spmd_kwargs)
    return _assemble(res.results), res


def kernel(x, size, theta):
    out, _ = run(x, size, theta, trace=False)
    return out
